# revision 16
# baseline (speedup 1.0000x reference)
"""Trainium2 Bass kernel: DecorrelationNormalization (IterNorm whitening).

Input  x: (64, 56, 56, 256) f32, gamma/beta: (1,1,1,256) f32.
Sharding: data-parallel over batch across 8 NeuronCores (8 batches/core).
Per-group (4 groups of 64 channels) covariance stats are computed locally
as uncentered second moments + channel sums, all-reduced (132KB), the tiny
Newton-Schulz iteration is replicated on every core, and the whitening
matmul is applied locally from a bf16 SBUF-resident transposed cache.
"""

import sys

for p in ("/opt/trn_rl_repo", "/opt/pypackages"):
    if p not in sys.path:
        sys.path.append(p)

import numpy as np

import concourse.bass as bass
import concourse.bacc as bacc
import concourse.tile as tile
from concourse import mybir
from concourse.bass_utils import run_bass_kernel_spmd

F32 = mybir.dt.float32
BF16 = mybir.dt.bfloat16

# Problem constants (hardcoded per spec).
B, H, W, C = 64, 56, 56, 256
NCORES = 8
BLOC = B // NCORES                    # 8 batches per core
NLOC = BLOC * H * W                   # 25088 positions per core
NGLOB = B * H * W                     # 200704 positions globally
CHUNK = 128                           # positions per chunk (partition dim)
NCHUNK = NLOC // CHUNK                # 196
SUP = 5                               # chunks per super-tile (DMA batch)
NSUP = (NCHUNK + SUP - 1) // SUP      # 40 (last super has 1 chunk)
EPS = 1e-5
ITER_NUM = 5

AOP = mybir.AluOpType
AFT = mybir.ActivationFunctionType


def build_bass() -> bass.Bass:
    nc = bacc.Bacc(None, num_devices=NCORES)

    x_d = nc.declare_dram_parameter("x", [BLOC, H, W, C], F32, isOutput=False)
    g_d = nc.declare_dram_parameter("gamma", [1, 1, 1, C], F32, isOutput=False)
    b_d = nc.declare_dram_parameter("beta", [1, 1, 1, C], F32, isOutput=False)
    eye_d = nc.declare_dram_parameter("eye", [128, 128], F32, isOutput=False)
    y_d = nc.declare_dram_parameter("out", [BLOC, H, W, C], F32, isOutput=True)

    xv = x_d[:].flatten_outer_dims()     # (25088, 256)
    yv = y_d[:].flatten_outer_dims()     # (25088, 256)
    gv = g_d[:].flatten_outer_dims()     # (1, 256)
    bv = b_d[:].flatten_outer_dims()     # (1, 256)

    with tile.TileContext(nc) as tc:
        with (
            tc.tile_pool(name="keep", bufs=1) as keep,
            tc.tile_pool(name="inp", bufs=8) as inp,
            tc.tile_pool(name="btp", bufs=4) as btp,
            tc.tile_pool(name="outp", bufs=8) as outp,
            tc.tile_pool(name="small", bufs=1) as small,
            tc.tile_pool(name="ps_acc", bufs=1, space="PSUM") as ps_acc,
            tc.tile_pool(name="ps_rot", bufs=3, space="PSUM") as ps_rot,
            tc.tile_pool(name="ps_rot2", bufs=3, space="PSUM") as ps_rot2,
            tc.tile_pool(name="dram", bufs=1, space="DRAM") as dram,
        ):
            # ---------------- constants ----------------
            eye_sb = keep.tile([128, 128], F32)
            nc.sync.dma_start(out=eye_sb[:], in_=eye_d[:])
            eye_bf = keep.tile([128, 128], BF16)
            nc.vector.tensor_copy(out=eye_bf[:], in_=eye_sb[:])
            eye15 = keep.tile([128, 128], F32)
            nc.vector.tensor_scalar_mul(eye15[:], eye_sb[:], 1.5)
            ones_f = keep.tile([1, 128], F32)
            nc.vector.memset(ones_f[:], 1.0)
            ones_bf = keep.tile([1, 128], BF16)
            nc.vector.memset(ones_bf[:], 1.0)
            gam_row = keep.tile([1, C], F32)
            nc.sync.dma_start(out=gam_row[:], in_=gv[:])
            bet_row = keep.tile([1, C], F32)
            nc.sync.dma_start(out=bet_row[:], in_=bv[:])

            # bf16 transposed cache: [channel, pair, position]
            XtAB = keep.tile([128, 2, NLOC], BF16)

            # --------------- pass 1: stats + transpose cache ---------------
            # Per chunk (128 positions): cast f32->bf16, then per channel
            # half: cov matmul with an embedded ones column (accumulating
            # second moments + channel sums in PSUM), and a plain matmul
            # against the identity producing the transposed tile.
            ps_cov01 = ps_acc.tile([128, 129], F32)
            ps_cov23 = ps_acc.tile([128, 129], F32)

            for s in range(NSUP):
                ns = min(SUP, NCHUNK - s * SUP)
                it = inp.tile([128, ns, 256], F32, tag="it")
                rows = xv[s * SUP * CHUNK:(s * SUP + ns) * CHUNK, :]
                rows = rows.rearrange("(c p) f -> p c f", p=128)
                nc.sync.dma_start(out=it[:], in_=rows[:])

                bt = btp.tile([128, ns, 260], BF16, tag="bt")
                nc.vector.memset(bt[:, :, 128:129], 1.0)
                nc.vector.memset(bt[:, :, 258:259], 1.0)

                for c in range(ns):
                    k = s * SUP + c
                    first = (k == 0)
                    last = (k == NCHUNK - 1)
                    nc.gpsimd.tensor_copy(out=bt[:, c, 0:128],
                                          in_=it[:, c, 0:128])
                    nc.gpsimd.tensor_copy(out=bt[:, c, 130:258],
                                          in_=it[:, c, 128:256])
                    t0 = bt[:, c, 0:128]
                    t1 = bt[:, c, 130:258]
                    if k % 2 == 0:
                        pot = ps_rot.tile([128, 256], F32, tag="rot")
                    else:
                        pot = ps_rot2.tile([128, 256], F32, tag="rot2")
                    nc.tensor.matmul(ps_cov01[:], t0, bt[:, c, 0:129],
                                     start=first, stop=last)
                    nc.tensor.matmul(pot[:, 0:128], t0, eye_bf[:],
                                     start=True, stop=True,
                                     skip_group_check=True)
                    nc.tensor.matmul(ps_cov23[:], t1, bt[:, c, 130:259],
                                     start=first, stop=last)
                    nc.tensor.matmul(pot[:, 128:256], t1, eye_bf[:],
                                     start=True, stop=True,
                                     skip_group_check=True)
                    dst = XtAB[:, :, k * CHUNK:(k + 1) * CHUNK]
                    if k % 2 == 0:
                        nc.vector.tensor_copy(out=dst, in_=pot[:])
                    else:
                        nc.scalar.copy(out=dst, in_=pot[:])

            # --------------- all-reduce the stats ---------------
            S_sb = keep.tile([128, 258], F32)
            nc.vector.tensor_copy(out=S_sb[:, 0:129], in_=ps_cov01[:])
            nc.vector.tensor_copy(out=S_sb[:, 129:258], in_=ps_cov23[:])

            bounce_in = dram.tile([128, 258], F32)
            bounce_out = dram.tile([128, 258], F32)
            nc.sync.dma_start(out=bounce_in[:], in_=S_sb[:])
            nc.gpsimd.collective_compute(
                "AllReduce",
                AOP.add,
                replica_groups=[list(range(NCORES))],
                ins=[bounce_in[:].opt()],
                outs=[bounce_out[:].opt()],
            )
            S_red = S_sb
            nc.sync.dma_start(out=S_red[:], in_=bounce_out[:])

            # --------------- replicated stats assembly + Newton-Schulz -----
            # Per pair: PS tile holds [P | sigma] as (128, 256).
            PS = [keep.tile([128, 256], F32, name=f"PS{p}", tag=f"PS{p}") for p in range(2)]
            mu = [keep.tile([128, 1], F32, name=f"mu{p}", tag=f"mu{p}") for p in range(2)]
            itr_col = [keep.tile([128, 1], F32, name=f"itr{p}", tag=f"itr{p}") for p in range(2)]
            rtr_col = [keep.tile([128, 1], F32, name=f"rtr{p}", tag=f"rtr{p}") for p in range(2)]
            trrow = keep.tile([1, 4], F32)

            a_coef = (1.0 - EPS) / (NGLOB - 1.0)
            b_coef = -(1.0 - EPS) * NGLOB / (NGLOB - 1.0)

            for p in range(2):
                cov = S_red[:, 129 * p:129 * p + 128]
                sums = S_red[:, 129 * p + 128:129 * p + 129]
                nc.vector.tensor_scalar_mul(mu[p][:], sums, 1.0 / NGLOB)
                # mu row via PE transpose
                ps_mur = ps_rot.tile([1, 128], F32, tag="rot")
                nc.tensor.transpose(ps_mur[:], mu[p][:], eye_sb[:])
                mur = small.tile([1, 128], F32, tag="rowtmp")
                nc.vector.tensor_copy(out=mur[:], in_=ps_mur[:])
                # mu mu^T diagonal blocks (64x64 each)
                ps_muu = ps_rot.tile([128, 64], F32, tag="rot")
                for gl in range(2):
                    nc.tensor.matmul(
                        ps_muu[64 * gl:64 * (gl + 1), 0:64],
                        mur[0:1, 64 * gl:64 * (gl + 1)],
                        mur[0:1, 64 * gl:64 * (gl + 1)],
                        start=True, stop=True,
                        tile_position=(0, 64 * gl),
                        skip_group_check=True,
                    )
                # sigma := (1-eps)*(S - N mu mu^T)/(N-1) + eps I, block-diag
                sig = PS[p][:, 128:256]
                nc.vector.memset(sig, 0.0)
                mt = small.tile([128, 64], F32, tag="mt")
                nc.vector.tensor_scalar_mul(mt[:], ps_muu[:], b_coef)
                for gl in range(2):
                    sblk = cov[64 * gl:64 * (gl + 1), 64 * gl:64 * (gl + 1)]
                    nc.vector.scalar_tensor_tensor(
                        out=PS[p][64 * gl:64 * (gl + 1),
                                  128 + 64 * gl:128 + 64 * (gl + 1)],
                        in0=sblk, scalar=a_coef,
                        in1=mt[64 * gl:64 * (gl + 1), :],
                        op0=AOP.mult, op1=AOP.add,
                    )
                nc.vector.scalar_tensor_tensor(
                    out=sig, in0=eye_sb[:], scalar=EPS, in1=sig,
                    op0=AOP.mult, op1=AOP.add)
                # traces of the two 64x64 blocks
                dt_full = small.tile([128, 256], F32, tag="scratch", name="dt_full")
                dt_ = dt_full[:, 0:128]
                nc.vector.tensor_mul(dt_, sig, eye_sb[:])
                dcol = small.tile([128, 1], F32, tag="dcol")
                nc.vector.tensor_reduce(dcol[:], dt_,
                                        axis=mybir.AxisListType.X, op=AOP.add)
                ps_dr = ps_rot.tile([1, 128], F32, tag="rot")
                nc.tensor.transpose(ps_dr[:], dcol[:], eye_sb[:])
                drow = small.tile([1, 128], F32, tag="rowtmp")
                nc.vector.tensor_copy(out=drow[:], in_=ps_dr[:])
                for gl in range(2):
                    nc.vector.tensor_reduce(
                        trrow[0:1, 2 * p + gl:2 * p + gl + 1],
                        drow[0:1, 64 * gl:64 * (gl + 1)],
                        axis=mybir.AxisListType.X, op=AOP.add)

            # 1/tr and 1/sqrt(tr) (+1 Newton-Raphson polish for rsqrt)
            itr_row = keep.tile([1, 4], F32)
            nc.vector.reciprocal(itr_row[:], trrow[:])
            rtr_row = keep.tile([1, 4], F32)
            sq_row = keep.tile([1, 4], F32)
            nc.scalar.activation(out=sq_row[:], in_=trrow[:], func=AFT.Sqrt)
            nc.vector.reciprocal(rtr_row[:], sq_row[:])
            nr = small.tile([1, 4], F32, tag="nr")
            nc.vector.tensor_mul(nr[:], rtr_row[:], rtr_row[:])
            nc.vector.tensor_mul(nr[:], nr[:], trrow[:])
            nc.vector.tensor_scalar(out=nr[:], in0=nr[:], scalar1=-0.5,
                                    scalar2=1.5, op0=AOP.mult, op1=AOP.add)
            nc.vector.tensor_mul(rtr_row[:], rtr_row[:], nr[:])

            # broadcast per-group scalars into per-partition columns
            for p in range(2):
                ps_itr = ps_rot.tile([128, 1], F32, tag="rot")
                ps_rtr = ps_rot.tile([128, 1], F32, tag="rot")
                for gl in range(2):
                    nc.tensor.matmul(
                        ps_itr[64 * gl:64 * (gl + 1), 0:1],
                        ones_f[0:1, 0:64],
                        itr_row[0:1, 2 * p + gl:2 * p + gl + 1],
                        start=True, stop=True, tile_position=(0, 64 * gl),
                        skip_group_check=True,
                    )
                    nc.tensor.matmul(
                        ps_rtr[64 * gl:64 * (gl + 1), 0:1],
                        ones_f[0:1, 0:64],
                        rtr_row[0:1, 2 * p + gl:2 * p + gl + 1],
                        start=True, stop=True, tile_position=(0, 64 * gl),
                        skip_group_check=True,
                    )
                nc.vector.tensor_copy(out=itr_col[p][:], in_=ps_itr[:])
                nc.vector.tensor_copy(out=rtr_col[p][:], in_=ps_rtr[:])
                # sigma /= trace ; P1 = 1.5 I - 0.5 sigma
                sig = PS[p][:, 128:256]
                nc.vector.tensor_scalar_mul(sig, sig, itr_col[p][:])
                nc.vector.scalar_tensor_tensor(
                    out=PS[p][:, 0:128], in0=sig, scalar=-0.5, in1=eye15[:],
                    op0=AOP.mult, op1=AOP.add)

            # Newton-Schulz iterations 2..5:
            #   [P^2 | P sigma] = P @ [P | sigma];  P' = 1.5 P - 0.5 P^2 (P sigma)
            for _ in range(ITER_NUM - 1):
                for p in range(2):
                    ps1 = ps_rot.tile([128, 256], F32, tag="rot")
                    nc.tensor.matmul(ps1[:], PS[p][:, 0:128], PS[p][:, 0:256],
                                     start=True, stop=True)
                    tmp = small.tile([128, 256], F32, tag="scratch")
                    nc.vector.tensor_copy(out=tmp[:], in_=ps1[:])
                    ps2 = ps_rot.tile([128, 128], F32, tag="rot")
                    nc.tensor.matmul(ps2[:], tmp[:, 0:128], tmp[:, 128:256],
                                     start=True, stop=True)
                    tP = small.tile([128, 128], F32, tag="tP")
                    nc.vector.tensor_scalar_mul(tP[:], PS[p][:, 0:128], 1.5)
                    nc.vector.scalar_tensor_tensor(
                        out=PS[p][:, 0:128], in0=ps2[:], scalar=-0.5,
                        in1=tP[:], op0=AOP.mult, op1=AOP.add)

            # W = (P / sqrt(tr)) * gamma_col ; bias = beta - mu^T W
            Wbf = [keep.tile([128, 128], BF16, name=f"Wbf{p}", tag=f"Wbf{p}") for p in range(2)]
            brow_f = keep.tile([1, C], F32)
            for p in range(2):
                wmf = small.tile([128, 128], F32, tag="wmf")
                nc.vector.tensor_scalar_mul(wmf[:], PS[p][:, 0:128],
                                            rtr_col[p][:])
                ps_g = ps_rot.tile([128, 128], F32, tag="rot")
                nc.tensor.matmul(ps_g[:], ones_f[0:1, 0:128],
                                 gam_row[0:1, 128 * p:128 * (p + 1)],
                                 start=True, stop=True)
                Wf = small.tile([128, 128], F32, tag="Wf")
                nc.vector.tensor_mul(Wf[:], wmf[:], ps_g[:])
                nc.vector.tensor_copy(out=Wbf[p][:], in_=Wf[:])
                ps_b = ps_rot.tile([1, 128], F32, tag="rot")
                nc.tensor.matmul(ps_b[:], mu[p][:], Wf[:],
                                 start=True, stop=True)
                nc.vector.scalar_tensor_tensor(
                    out=brow_f[0:1, 128 * p:128 * (p + 1)], in0=ps_b[:],
                    scalar=-1.0, in1=bet_row[0:1, 128 * p:128 * (p + 1)],
                    op0=AOP.mult, op1=AOP.add)
            brow_bf = keep.tile([1, C], BF16)
            nc.vector.tensor_copy(out=brow_bf[:], in_=brow_f[:])

            # --------------- pass 2: whiten ---------------
            for s in range(NSUP):
                ns = min(SUP, NCHUNK - s * SUP)
                ot = outp.tile([128, ns, C], F32, tag="ot")
                for c in range(ns):
                    k = s * SUP + c
                    if k % 2 == 0:
                        po = ps_rot.tile([128, 256], F32, tag="rot")
                    else:
                        po = ps_rot2.tile([128, 256], F32, tag="rot2")
                    nc.tensor.matmul(po[:], ones_bf[0:1, 0:128], brow_bf[:],
                                     start=True, stop=False,
                                     skip_group_check=True)
                    nc.tensor.matmul(po[:, 0:128],
                                     XtAB[:, 0, k * CHUNK:(k + 1) * CHUNK],
                                     Wbf[0][:], start=False, stop=True,
                                     skip_group_check=True)
                    nc.tensor.matmul(po[:, 128:256],
                                     XtAB[:, 1, k * CHUNK:(k + 1) * CHUNK],
                                     Wbf[1][:], start=False, stop=True,
                                     skip_group_check=True)
                    if k % 2 == 0:
                        nc.vector.tensor_copy(out=ot[:, c, :], in_=po[:])
                    else:
                        nc.scalar.copy(out=ot[:, c, :], in_=po[:])
                orows = yv[s * SUP * CHUNK:(s * SUP + ns) * CHUNK, :]
                orows = orows.rearrange("(c p) f -> p c f", p=128)
                nc.sync.dma_start(out=orows[:], in_=ot[:])

    nc.finalize()
    return nc


_NC_CACHE = None


def _get_nc():
    global _NC_CACHE
    if _NC_CACHE is None:
        _NC_CACHE = build_bass()
    return _NC_CACHE


def make_in_maps(x, gamma, beta):
    x = np.ascontiguousarray(np.asarray(x, dtype=np.float32))
    gamma = np.asarray(gamma, dtype=np.float32)
    beta = np.asarray(beta, dtype=np.float32)
    eye = np.eye(128, dtype=np.float32)
    maps = []
    for i in range(NCORES):
        maps.append({
            "x": np.ascontiguousarray(x[i * BLOC:(i + 1) * BLOC]),
            "gamma": gamma,
            "beta": beta,
            "eye": eye,
        })
    return maps


def kernel(x, gamma, beta):
    nc = _get_nc()
    in_maps = make_in_maps(x, gamma, beta)
    res = run_bass_kernel_spmd(nc, in_maps, core_ids=list(range(NCORES)))
    out = np.concatenate([res.results[i]["out"] for i in range(NCORES)],
                         axis=0)
    return out.astype(np.float32)


if __name__ == "__main__":
    nc = build_bass()
    print("graph built OK")


# revision 17
# speedup vs baseline: 1.1677x; 1.1677x over previous
"""Trainium2 Bass kernel: DecorrelationNormalization (IterNorm whitening).

Input  x: (64, 56, 56, 256) f32, gamma/beta: (1,1,1,256) f32.
Sharding: data-parallel over batch across 8 NeuronCores (8 batches/core).
Per-group (4 groups of 64 channels) covariance stats are computed locally
as uncentered second moments + channel sums, all-reduced (132KB), the tiny
Newton-Schulz iteration is replicated on every core, and the whitening
matmul is applied locally from a bf16 SBUF-resident transposed cache.
"""

import sys

for p in ("/opt/trn_rl_repo", "/opt/pypackages"):
    if p not in sys.path:
        sys.path.append(p)

import numpy as np

import concourse.bass as bass
import concourse.bacc as bacc
import concourse.tile as tile
from concourse import mybir
from concourse.bass_utils import run_bass_kernel_spmd

F32 = mybir.dt.float32
BF16 = mybir.dt.bfloat16

# Problem constants (hardcoded per spec).
B, H, W, C = 64, 56, 56, 256
NCORES = 8
BLOC = B // NCORES                    # 8 batches per core
NLOC = BLOC * H * W                   # 25088 positions per core
NGLOB = B * H * W                     # 200704 positions globally
CHUNK = 128                           # positions per chunk (partition dim)
NCHUNK = NLOC // CHUNK                # 196
SUP = 5                               # chunks per super-tile (DMA batch)
NSUP = (NCHUNK + SUP - 1) // SUP      # 40 (last super has 1 chunk)
EPS = 1e-5
ITER_NUM = 5

AOP = mybir.AluOpType
AFT = mybir.ActivationFunctionType


def build_bass() -> bass.Bass:
    nc = bacc.Bacc(None, num_devices=NCORES)

    x_d = nc.declare_dram_parameter("x", [BLOC, H, W, C], F32, isOutput=False)
    g_d = nc.declare_dram_parameter("gamma", [1, 1, 1, C], F32, isOutput=False)
    b_d = nc.declare_dram_parameter("beta", [1, 1, 1, C], F32, isOutput=False)
    eye_d = nc.declare_dram_parameter("eye", [128, 128], F32, isOutput=False)
    y_d = nc.declare_dram_parameter("out", [BLOC, H, W, C], F32, isOutput=True)

    xv = x_d[:].flatten_outer_dims()     # (25088, 256)
    yv = y_d[:].flatten_outer_dims()     # (25088, 256)
    gv = g_d[:].flatten_outer_dims()     # (1, 256)
    bv = b_d[:].flatten_outer_dims()     # (1, 256)

    with tile.TileContext(nc) as tc:
        with (
            tc.tile_pool(name="keep", bufs=1) as keep,
            tc.tile_pool(name="inp", bufs=8) as inp,
            tc.tile_pool(name="btp", bufs=4) as btp,
            tc.tile_pool(name="outp", bufs=8) as outp,
            tc.tile_pool(name="small", bufs=1) as small,
            tc.tile_pool(name="ps_acc", bufs=1, space="PSUM") as ps_acc,
            tc.tile_pool(name="ps_rot", bufs=3, space="PSUM") as ps_rot,
            tc.tile_pool(name="ps_rot2", bufs=3, space="PSUM") as ps_rot2,
            tc.tile_pool(name="dram", bufs=1, space="DRAM") as dram,
        ):
            # ---------------- constants ----------------
            eye_sb = keep.tile([128, 128], F32)
            nc.sync.dma_start(out=eye_sb[:], in_=eye_d[:])
            eye_bf = keep.tile([128, 128], BF16)
            nc.vector.tensor_copy(out=eye_bf[:], in_=eye_sb[:])
            eye15 = keep.tile([128, 128], F32)
            nc.vector.tensor_scalar_mul(eye15[:], eye_sb[:], 1.5)
            ones_f = keep.tile([1, 128], F32)
            nc.vector.memset(ones_f[:], 1.0)
            ones_bf = keep.tile([1, 128], BF16)
            nc.vector.memset(ones_bf[:], 1.0)
            gam_row = keep.tile([1, C], F32)
            nc.sync.dma_start(out=gam_row[:], in_=gv[:])
            bet_row = keep.tile([1, C], F32)
            nc.sync.dma_start(out=bet_row[:], in_=bv[:])

            # bf16 transposed cache: [channel, pair, position]
            XtAB = keep.tile([128, 2, NLOC], BF16)

            # --------------- pass 1: stats + transpose cache ---------------
            # Per chunk (128 positions): cast f32->bf16, then per channel
            # half: cov matmul with an embedded ones column (accumulating
            # second moments + channel sums in PSUM), and a plain matmul
            # against the identity producing the transposed tile.
            ps_cov01 = ps_acc.tile([128, 129], F32)
            ps_cov23 = ps_acc.tile([128, 129], F32)

            for s in range(NSUP):
                ns = min(SUP, NCHUNK - s * SUP)
                it = inp.tile([128, ns, 256], F32, tag="it")
                rows = xv[s * SUP * CHUNK:(s * SUP + ns) * CHUNK, :]
                rows = rows.rearrange("(c p) f -> p c f", p=128)
                nc.gpsimd.dma_start(out=it[:], in_=rows[:])

                bt = btp.tile([128, ns, 260], BF16, tag="bt")
                nc.vector.memset(bt[:, :, 128:129], 1.0)
                nc.vector.memset(bt[:, :, 258:259], 1.0)
                nc.vector.tensor_scalar_mul(bt[:, :, 0:128],
                                            it[:, :, 0:128], 1.0)
                nc.vector.tensor_scalar_mul(bt[:, :, 130:258],
                                            it[:, :, 128:256], 1.0)

                for c in range(ns):
                    k = s * SUP + c
                    first = (k == 0)
                    last = (k == NCHUNK - 1)
                    t0 = bt[:, c, 0:128]
                    t1 = bt[:, c, 130:258]
                    if k % 2 == 0:
                        pot = ps_rot.tile([128, 256], F32, tag="rot")
                    else:
                        pot = ps_rot2.tile([128, 256], F32, tag="rot2")
                    nc.tensor.matmul(ps_cov01[:], t0, bt[:, c, 0:129],
                                     start=first, stop=last)
                    nc.tensor.matmul(pot[:, 0:128], t0, eye_bf[:],
                                     start=True, stop=True,
                                     skip_group_check=True)
                    nc.tensor.matmul(ps_cov23[:], t1, bt[:, c, 130:259],
                                     start=first, stop=last)
                    nc.tensor.matmul(pot[:, 128:256], t1, eye_bf[:],
                                     start=True, stop=True,
                                     skip_group_check=True)
                    dst = XtAB[:, :, k * CHUNK:(k + 1) * CHUNK]
                    if k % 2 == 0:
                        nc.vector.tensor_copy(out=dst, in_=pot[:])
                    else:
                        nc.scalar.copy(out=dst, in_=pot[:])

            # --------------- all-reduce the stats ---------------
            S_sb = keep.tile([128, 258], F32)
            nc.vector.tensor_copy(out=S_sb[:, 0:129], in_=ps_cov01[:])
            nc.vector.tensor_copy(out=S_sb[:, 129:258], in_=ps_cov23[:])

            bounce_in = dram.tile([128, 258], F32)
            bounce_out = dram.tile([128, 258], F32)
            nc.sync.dma_start(out=bounce_in[:], in_=S_sb[:])
            nc.gpsimd.collective_compute(
                "AllReduce",
                AOP.add,
                replica_groups=[list(range(NCORES))],
                ins=[bounce_in[:].opt()],
                outs=[bounce_out[:].opt()],
            )
            S_red = S_sb
            nc.sync.dma_start(out=S_red[:], in_=bounce_out[:])

            # --------------- replicated stats assembly + Newton-Schulz -----
            # Per pair: PS tile holds [P | sigma] as (128, 256).
            PS = [keep.tile([128, 256], F32, name=f"PS{p}", tag=f"PS{p}") for p in range(2)]
            mu = [keep.tile([128, 1], F32, name=f"mu{p}", tag=f"mu{p}") for p in range(2)]
            itr_col = [keep.tile([128, 1], F32, name=f"itr{p}", tag=f"itr{p}") for p in range(2)]
            rtr_col = [keep.tile([128, 1], F32, name=f"rtr{p}", tag=f"rtr{p}") for p in range(2)]
            trrow = keep.tile([1, 4], F32)

            a_coef = (1.0 - EPS) / (NGLOB - 1.0)
            b_coef = -(1.0 - EPS) * NGLOB / (NGLOB - 1.0)

            for p in range(2):
                cov = S_red[:, 129 * p:129 * p + 128]
                sums = S_red[:, 129 * p + 128:129 * p + 129]
                nc.vector.tensor_scalar_mul(mu[p][:], sums, 1.0 / NGLOB)
                # mu row via PE transpose
                ps_mur = ps_rot.tile([1, 128], F32, tag="rot")
                nc.tensor.transpose(ps_mur[:], mu[p][:], eye_sb[:])
                mur = small.tile([1, 128], F32, tag="rowtmp")
                nc.vector.tensor_copy(out=mur[:], in_=ps_mur[:])
                # mu mu^T diagonal blocks (64x64 each)
                ps_muu = ps_rot.tile([128, 64], F32, tag="rot")
                for gl in range(2):
                    nc.tensor.matmul(
                        ps_muu[64 * gl:64 * (gl + 1), 0:64],
                        mur[0:1, 64 * gl:64 * (gl + 1)],
                        mur[0:1, 64 * gl:64 * (gl + 1)],
                        start=True, stop=True,
                        tile_position=(0, 64 * gl),
                        skip_group_check=True,
                    )
                # sigma := (1-eps)*(S - N mu mu^T)/(N-1) + eps I, block-diag
                sig = PS[p][:, 128:256]
                nc.vector.memset(sig, 0.0)
                mt = small.tile([128, 64], F32, tag="mt")
                nc.vector.tensor_scalar_mul(mt[:], ps_muu[:], b_coef)
                for gl in range(2):
                    sblk = cov[64 * gl:64 * (gl + 1), 64 * gl:64 * (gl + 1)]
                    nc.vector.scalar_tensor_tensor(
                        out=PS[p][64 * gl:64 * (gl + 1),
                                  128 + 64 * gl:128 + 64 * (gl + 1)],
                        in0=sblk, scalar=a_coef,
                        in1=mt[64 * gl:64 * (gl + 1), :],
                        op0=AOP.mult, op1=AOP.add,
                    )
                nc.vector.scalar_tensor_tensor(
                    out=sig, in0=eye_sb[:], scalar=EPS, in1=sig,
                    op0=AOP.mult, op1=AOP.add)
                # traces of the two 64x64 blocks
                dt_full = small.tile([128, 256], F32, tag="scratch", name="dt_full")
                dt_ = dt_full[:, 0:128]
                nc.vector.tensor_mul(dt_, sig, eye_sb[:])
                dcol = small.tile([128, 1], F32, tag="dcol")
                nc.vector.tensor_reduce(dcol[:], dt_,
                                        axis=mybir.AxisListType.X, op=AOP.add)
                ps_dr = ps_rot.tile([1, 128], F32, tag="rot")
                nc.tensor.transpose(ps_dr[:], dcol[:], eye_sb[:])
                drow = small.tile([1, 128], F32, tag="rowtmp")
                nc.vector.tensor_copy(out=drow[:], in_=ps_dr[:])
                for gl in range(2):
                    nc.vector.tensor_reduce(
                        trrow[0:1, 2 * p + gl:2 * p + gl + 1],
                        drow[0:1, 64 * gl:64 * (gl + 1)],
                        axis=mybir.AxisListType.X, op=AOP.add)

            # 1/tr and 1/sqrt(tr) (+1 Newton-Raphson polish for rsqrt)
            itr_row = keep.tile([1, 4], F32)
            nc.vector.reciprocal(itr_row[:], trrow[:])
            rtr_row = keep.tile([1, 4], F32)
            sq_row = keep.tile([1, 4], F32)
            nc.scalar.activation(out=sq_row[:], in_=trrow[:], func=AFT.Sqrt)
            nc.vector.reciprocal(rtr_row[:], sq_row[:])
            nr = small.tile([1, 4], F32, tag="nr")
            nc.vector.tensor_mul(nr[:], rtr_row[:], rtr_row[:])
            nc.vector.tensor_mul(nr[:], nr[:], trrow[:])
            nc.vector.tensor_scalar(out=nr[:], in0=nr[:], scalar1=-0.5,
                                    scalar2=1.5, op0=AOP.mult, op1=AOP.add)
            nc.vector.tensor_mul(rtr_row[:], rtr_row[:], nr[:])

            # broadcast per-group scalars into per-partition columns
            for p in range(2):
                ps_itr = ps_rot.tile([128, 1], F32, tag="rot")
                ps_rtr = ps_rot.tile([128, 1], F32, tag="rot")
                for gl in range(2):
                    nc.tensor.matmul(
                        ps_itr[64 * gl:64 * (gl + 1), 0:1],
                        ones_f[0:1, 0:64],
                        itr_row[0:1, 2 * p + gl:2 * p + gl + 1],
                        start=True, stop=True, tile_position=(0, 64 * gl),
                        skip_group_check=True,
                    )
                    nc.tensor.matmul(
                        ps_rtr[64 * gl:64 * (gl + 1), 0:1],
                        ones_f[0:1, 0:64],
                        rtr_row[0:1, 2 * p + gl:2 * p + gl + 1],
                        start=True, stop=True, tile_position=(0, 64 * gl),
                        skip_group_check=True,
                    )
                nc.vector.tensor_copy(out=itr_col[p][:], in_=ps_itr[:])
                nc.vector.tensor_copy(out=rtr_col[p][:], in_=ps_rtr[:])
                # sigma /= trace ; P1 = 1.5 I - 0.5 sigma
                sig = PS[p][:, 128:256]
                nc.vector.tensor_scalar_mul(sig, sig, itr_col[p][:])
                nc.vector.scalar_tensor_tensor(
                    out=PS[p][:, 0:128], in0=sig, scalar=-0.5, in1=eye15[:],
                    op0=AOP.mult, op1=AOP.add)

            # Newton-Schulz iterations 2..5:
            #   [P^2 | P sigma] = P @ [P | sigma];  P' = 1.5 P - 0.5 P^2 (P sigma)
            for _ in range(ITER_NUM - 1):
                for p in range(2):
                    ps1 = ps_rot.tile([128, 256], F32, tag="rot")
                    nc.tensor.matmul(ps1[:], PS[p][:, 0:128], PS[p][:, 0:256],
                                     start=True, stop=True)
                    tmp = small.tile([128, 256], F32, tag="scratch")
                    nc.vector.tensor_copy(out=tmp[:], in_=ps1[:])
                    ps2 = ps_rot.tile([128, 128], F32, tag="rot")
                    nc.tensor.matmul(ps2[:], tmp[:, 0:128], tmp[:, 128:256],
                                     start=True, stop=True)
                    tP = small.tile([128, 128], F32, tag="tP")
                    nc.vector.tensor_scalar_mul(tP[:], PS[p][:, 0:128], 1.5)
                    nc.vector.scalar_tensor_tensor(
                        out=PS[p][:, 0:128], in0=ps2[:], scalar=-0.5,
                        in1=tP[:], op0=AOP.mult, op1=AOP.add)

            # W = (P / sqrt(tr)) * gamma_col ; bias = beta - mu^T W
            Wbf = [keep.tile([128, 128], BF16, name=f"Wbf{p}", tag=f"Wbf{p}") for p in range(2)]
            brow_f = keep.tile([1, C], F32)
            for p in range(2):
                wmf = small.tile([128, 128], F32, tag="wmf")
                nc.vector.tensor_scalar_mul(wmf[:], PS[p][:, 0:128],
                                            rtr_col[p][:])
                ps_g = ps_rot.tile([128, 128], F32, tag="rot")
                nc.tensor.matmul(ps_g[:], ones_f[0:1, 0:128],
                                 gam_row[0:1, 128 * p:128 * (p + 1)],
                                 start=True, stop=True)
                Wf = small.tile([128, 128], F32, tag="Wf")
                nc.vector.tensor_mul(Wf[:], wmf[:], ps_g[:])
                nc.vector.tensor_copy(out=Wbf[p][:], in_=Wf[:])
                ps_b = ps_rot.tile([1, 128], F32, tag="rot")
                nc.tensor.matmul(ps_b[:], mu[p][:], Wf[:],
                                 start=True, stop=True)
                nc.vector.scalar_tensor_tensor(
                    out=brow_f[0:1, 128 * p:128 * (p + 1)], in0=ps_b[:],
                    scalar=-1.0, in1=bet_row[0:1, 128 * p:128 * (p + 1)],
                    op0=AOP.mult, op1=AOP.add)
            brow_bf = keep.tile([1, C], BF16)
            nc.vector.tensor_copy(out=brow_bf[:], in_=brow_f[:])

            # --------------- pass 2: whiten ---------------
            for s in range(NSUP):
                ns = min(SUP, NCHUNK - s * SUP)
                ot = outp.tile([128, ns, C], F32, tag="ot")
                for c in range(ns):
                    k = s * SUP + c
                    if k % 2 == 0:
                        po = ps_rot.tile([128, 256], F32, tag="rot")
                    else:
                        po = ps_rot2.tile([128, 256], F32, tag="rot2")
                    nc.tensor.matmul(po[:], ones_bf[0:1, 0:128], brow_bf[:],
                                     start=True, stop=False,
                                     skip_group_check=True)
                    nc.tensor.matmul(po[:, 0:128],
                                     XtAB[:, 0, k * CHUNK:(k + 1) * CHUNK],
                                     Wbf[0][:], start=False, stop=True,
                                     skip_group_check=True)
                    nc.tensor.matmul(po[:, 128:256],
                                     XtAB[:, 1, k * CHUNK:(k + 1) * CHUNK],
                                     Wbf[1][:], start=False, stop=True,
                                     skip_group_check=True)
                    if k % 2 == 0:
                        nc.vector.tensor_copy(out=ot[:, c, :], in_=po[:])
                    else:
                        nc.scalar.copy(out=ot[:, c, :], in_=po[:])
                orows = yv[s * SUP * CHUNK:(s * SUP + ns) * CHUNK, :]
                orows = orows.rearrange("(c p) f -> p c f", p=128)
                nc.gpsimd.dma_start(out=orows[:], in_=ot[:])

    nc.finalize()
    return nc


_NC_CACHE = None


def _get_nc():
    global _NC_CACHE
    if _NC_CACHE is None:
        _NC_CACHE = build_bass()
    return _NC_CACHE


def make_in_maps(x, gamma, beta):
    x = np.ascontiguousarray(np.asarray(x, dtype=np.float32))
    gamma = np.asarray(gamma, dtype=np.float32)
    beta = np.asarray(beta, dtype=np.float32)
    eye = np.eye(128, dtype=np.float32)
    maps = []
    for i in range(NCORES):
        maps.append({
            "x": np.ascontiguousarray(x[i * BLOC:(i + 1) * BLOC]),
            "gamma": gamma,
            "beta": beta,
            "eye": eye,
        })
    return maps


def kernel(x, gamma, beta):
    nc = _get_nc()
    in_maps = make_in_maps(x, gamma, beta)
    res = run_bass_kernel_spmd(nc, in_maps, core_ids=list(range(NCORES)))
    out = np.concatenate([res.results[i]["out"] for i in range(NCORES)],
                         axis=0)
    return out.astype(np.float32)


if __name__ == "__main__":
    nc = build_bass()
    print("graph built OK")


# revision 18
# speedup vs baseline: 1.5224x; 1.3038x over previous
"""Trainium2 Bass kernel: DecorrelationNormalization (IterNorm whitening).

Input  x: (64, 56, 56, 256) f32, gamma/beta: (1,1,1,256) f32.
Sharding: data-parallel over batch across 8 NeuronCores (8 batches/core).
Per-group (4 groups of 64 channels) covariance stats are computed locally
as uncentered second moments + channel sums, all-reduced (132KB), the tiny
Newton-Schulz iteration is replicated on every core, and the whitening
matmul is applied locally from a bf16 SBUF-resident transposed cache.
"""

import sys

for p in ("/opt/trn_rl_repo", "/opt/pypackages"):
    if p not in sys.path:
        sys.path.append(p)

import numpy as np

import concourse.bass as bass
import concourse.bacc as bacc
import concourse.tile as tile
from concourse import mybir
from concourse.bass_utils import run_bass_kernel_spmd

F32 = mybir.dt.float32
BF16 = mybir.dt.bfloat16

# Problem constants (hardcoded per spec).
B, H, W, C = 64, 56, 56, 256
NCORES = 8
BLOC = B // NCORES                    # 8 batches per core
NLOC = BLOC * H * W                   # 25088 positions per core
NGLOB = B * H * W                     # 200704 positions globally
CHUNK = 128                           # positions per chunk (partition dim)
NCHUNK = NLOC // CHUNK                # 196
SUP = 5                               # chunks per super-tile (DMA batch)
NSUP = (NCHUNK + SUP - 1) // SUP      # 40 (last super has 1 chunk)
EPS = 1e-5
ITER_NUM = 5

AOP = mybir.AluOpType
AFT = mybir.ActivationFunctionType


def build_bass() -> bass.Bass:
    nc = bacc.Bacc(None, num_devices=NCORES)

    x_d = nc.declare_dram_parameter("x", [BLOC, H, W, C], F32, isOutput=False)
    g_d = nc.declare_dram_parameter("gamma", [1, 1, 1, C], F32, isOutput=False)
    b_d = nc.declare_dram_parameter("beta", [1, 1, 1, C], F32, isOutput=False)
    eye_d = nc.declare_dram_parameter("eye", [128, 128], F32, isOutput=False)
    y_d = nc.declare_dram_parameter("out", [BLOC, H, W, C], F32, isOutput=True)

    xv = x_d[:].flatten_outer_dims()     # (25088, 256)
    yv = y_d[:].flatten_outer_dims()     # (25088, 256)
    gv = g_d[:].flatten_outer_dims()     # (1, 256)
    bv = b_d[:].flatten_outer_dims()     # (1, 256)

    with tile.TileContext(nc) as tc:
        with (
            tc.tile_pool(name="keep", bufs=1) as keep,
            tc.tile_pool(name="inp", bufs=8) as inp,
            tc.tile_pool(name="btp", bufs=4) as btp,
            tc.tile_pool(name="outp", bufs=8) as outp,
            tc.tile_pool(name="small", bufs=1) as small,
            tc.tile_pool(name="ps_acc", bufs=1, space="PSUM") as ps_acc,
            tc.tile_pool(name="ps_rot", bufs=3, space="PSUM") as ps_rot,
            tc.tile_pool(name="ps_rot2", bufs=3, space="PSUM") as ps_rot2,
            tc.tile_pool(name="dram", bufs=1, space="DRAM") as dram,
        ):
            # ---------------- constants ----------------
            eye_sb = keep.tile([128, 128], F32)
            nc.sync.dma_start(out=eye_sb[:], in_=eye_d[:])
            eye_bf = keep.tile([128, 128], BF16)
            nc.vector.tensor_copy(out=eye_bf[:], in_=eye_sb[:])
            eye15 = keep.tile([128, 128], F32)
            nc.vector.tensor_scalar_mul(eye15[:], eye_sb[:], 1.5)
            ones_f = keep.tile([1, 128], F32)
            nc.vector.memset(ones_f[:], 1.0)
            ones_bf = keep.tile([1, 128], BF16)
            nc.vector.memset(ones_bf[:], 1.0)
            gam_row = keep.tile([1, C], F32)
            nc.sync.dma_start(out=gam_row[:], in_=gv[:])
            bet_row = keep.tile([1, C], F32)
            nc.sync.dma_start(out=bet_row[:], in_=bv[:])

            # bf16 transposed cache: [channel, pair, position]
            XtAB = keep.tile([128, 2, NLOC], BF16)

            # --------------- pass 1: stats + transpose cache ---------------
            # Per chunk (128 positions): cast f32->bf16, then per channel
            # half: cov matmul with an embedded ones column (accumulating
            # second moments + channel sums in PSUM), and a plain matmul
            # against the identity producing the transposed tile.
            ps_cov01 = ps_acc.tile([128, 129], F32)
            ps_cov23 = ps_acc.tile([128, 129], F32)

            for s in range(NSUP):
                ns = min(SUP, NCHUNK - s * SUP)
                it = inp.tile([128, ns, 256], F32, tag="it")
                rows = xv[s * SUP * CHUNK:(s * SUP + ns) * CHUNK, :]
                rows = rows.rearrange("(c p) f -> p c f", p=128)
                nc.gpsimd.dma_start(out=it[:], in_=rows[:])

                bt = btp.tile([128, ns, 260], BF16, tag="bt")
                nc.gpsimd.memset(bt[:, :, 128:129], 1.0)
                nc.gpsimd.memset(bt[:, :, 258:259], 1.0)
                nc.vector.tensor_scalar_mul(bt[:, :, 0:128],
                                            it[:, :, 0:128], 1.0)
                nc.vector.tensor_scalar_mul(bt[:, :, 130:258],
                                            it[:, :, 128:256], 1.0)

                for c in range(ns):
                    k = s * SUP + c
                    first = (k == 0)
                    last = (k == NCHUNK - 1)
                    t0 = bt[:, c, 0:128]
                    t1 = bt[:, c, 130:258]
                    if k % 4 == 0:
                        pot = ps_rot.tile([128, 256], F32, tag="rot")
                    else:
                        pot = ps_rot2.tile([128, 256], F32, tag="rot2")
                    nc.tensor.matmul(ps_cov01[:], t0, bt[:, c, 0:129],
                                     start=first, stop=last)
                    nc.tensor.matmul(pot[:, 0:128], t0, eye_bf[:],
                                     start=True, stop=True,
                                     skip_group_check=True)
                    nc.tensor.matmul(ps_cov23[:], t1, bt[:, c, 130:259],
                                     start=first, stop=last)
                    nc.tensor.matmul(pot[:, 128:256], t1, eye_bf[:],
                                     start=True, stop=True,
                                     skip_group_check=True)
                    dst = XtAB[:, :, k * CHUNK:(k + 1) * CHUNK]
                    if k % 4 == 0:
                        nc.vector.tensor_copy(out=dst, in_=pot[:])
                    else:
                        nc.scalar.copy(out=dst, in_=pot[:])

            # --------------- all-reduce the stats ---------------
            S_sb = keep.tile([128, 258], F32)
            nc.vector.tensor_copy(out=S_sb[:, 0:129], in_=ps_cov01[:])
            nc.vector.tensor_copy(out=S_sb[:, 129:258], in_=ps_cov23[:])

            bounce_in = dram.tile([128, 258], F32)
            bounce_out = dram.tile([128, 258], F32)
            nc.sync.dma_start(out=bounce_in[:], in_=S_sb[:])
            nc.gpsimd.collective_compute(
                "AllReduce",
                AOP.add,
                replica_groups=[list(range(NCORES))],
                ins=[bounce_in[:].opt()],
                outs=[bounce_out[:].opt()],
            )
            S_red = S_sb
            nc.sync.dma_start(out=S_red[:], in_=bounce_out[:])

            # --------------- replicated stats assembly + Newton-Schulz -----
            # Per pair: PS tile holds [P | sigma] as (128, 256).
            PS = [keep.tile([128, 256], F32, name=f"PS{p}", tag=f"PS{p}") for p in range(2)]
            mu = [keep.tile([128, 1], F32, name=f"mu{p}", tag=f"mu{p}") for p in range(2)]
            itr_col = [keep.tile([128, 1], F32, name=f"itr{p}", tag=f"itr{p}") for p in range(2)]
            rtr_col = [keep.tile([128, 1], F32, name=f"rtr{p}", tag=f"rtr{p}") for p in range(2)]
            trrow = keep.tile([1, 4], F32)

            a_coef = (1.0 - EPS) / (NGLOB - 1.0)
            b_coef = -(1.0 - EPS) * NGLOB / (NGLOB - 1.0)

            for p in range(2):
                cov = S_red[:, 129 * p:129 * p + 128]
                sums = S_red[:, 129 * p + 128:129 * p + 129]
                nc.vector.tensor_scalar_mul(mu[p][:], sums, 1.0 / NGLOB)
                # mu row via PE transpose
                ps_mur = ps_rot.tile([1, 128], F32, tag="rot")
                nc.tensor.transpose(ps_mur[:], mu[p][:], eye_sb[:])
                mur = small.tile([1, 128], F32, tag="rowtmp")
                nc.vector.tensor_copy(out=mur[:], in_=ps_mur[:])
                # mu mu^T diagonal blocks (64x64 each)
                ps_muu = ps_rot.tile([128, 64], F32, tag="rot")
                for gl in range(2):
                    nc.tensor.matmul(
                        ps_muu[64 * gl:64 * (gl + 1), 0:64],
                        mur[0:1, 64 * gl:64 * (gl + 1)],
                        mur[0:1, 64 * gl:64 * (gl + 1)],
                        start=True, stop=True,
                        tile_position=(0, 64 * gl),
                        skip_group_check=True,
                    )
                # sigma := (1-eps)*(S - N mu mu^T)/(N-1) + eps I, block-diag
                sig = PS[p][:, 128:256]
                nc.vector.memset(sig, 0.0)
                mt = small.tile([128, 64], F32, tag="mt")
                nc.vector.tensor_scalar_mul(mt[:], ps_muu[:], b_coef)
                for gl in range(2):
                    sblk = cov[64 * gl:64 * (gl + 1), 64 * gl:64 * (gl + 1)]
                    nc.vector.scalar_tensor_tensor(
                        out=PS[p][64 * gl:64 * (gl + 1),
                                  128 + 64 * gl:128 + 64 * (gl + 1)],
                        in0=sblk, scalar=a_coef,
                        in1=mt[64 * gl:64 * (gl + 1), :],
                        op0=AOP.mult, op1=AOP.add,
                    )
                nc.vector.scalar_tensor_tensor(
                    out=sig, in0=eye_sb[:], scalar=EPS, in1=sig,
                    op0=AOP.mult, op1=AOP.add)
                # traces of the two 64x64 blocks
                dt_full = small.tile([128, 256], F32, tag="scratch", name="dt_full")
                dt_ = dt_full[:, 0:128]
                nc.vector.tensor_mul(dt_, sig, eye_sb[:])
                dcol = small.tile([128, 1], F32, tag="dcol")
                nc.vector.tensor_reduce(dcol[:], dt_,
                                        axis=mybir.AxisListType.X, op=AOP.add)
                ps_dr = ps_rot.tile([1, 128], F32, tag="rot")
                nc.tensor.transpose(ps_dr[:], dcol[:], eye_sb[:])
                drow = small.tile([1, 128], F32, tag="rowtmp")
                nc.vector.tensor_copy(out=drow[:], in_=ps_dr[:])
                for gl in range(2):
                    nc.vector.tensor_reduce(
                        trrow[0:1, 2 * p + gl:2 * p + gl + 1],
                        drow[0:1, 64 * gl:64 * (gl + 1)],
                        axis=mybir.AxisListType.X, op=AOP.add)

            # 1/tr and 1/sqrt(tr) (+1 Newton-Raphson polish for rsqrt)
            itr_row = keep.tile([1, 4], F32)
            nc.vector.reciprocal(itr_row[:], trrow[:])
            rtr_row = keep.tile([1, 4], F32)
            sq_row = keep.tile([1, 4], F32)
            nc.scalar.activation(out=sq_row[:], in_=trrow[:], func=AFT.Sqrt)
            nc.vector.reciprocal(rtr_row[:], sq_row[:])
            nr = small.tile([1, 4], F32, tag="nr")
            nc.vector.tensor_mul(nr[:], rtr_row[:], rtr_row[:])
            nc.vector.tensor_mul(nr[:], nr[:], trrow[:])
            nc.vector.tensor_scalar(out=nr[:], in0=nr[:], scalar1=-0.5,
                                    scalar2=1.5, op0=AOP.mult, op1=AOP.add)
            nc.vector.tensor_mul(rtr_row[:], rtr_row[:], nr[:])

            # broadcast per-group scalars into per-partition columns
            for p in range(2):
                ps_itr = ps_rot.tile([128, 1], F32, tag="rot")
                ps_rtr = ps_rot.tile([128, 1], F32, tag="rot")
                for gl in range(2):
                    nc.tensor.matmul(
                        ps_itr[64 * gl:64 * (gl + 1), 0:1],
                        ones_f[0:1, 0:64],
                        itr_row[0:1, 2 * p + gl:2 * p + gl + 1],
                        start=True, stop=True, tile_position=(0, 64 * gl),
                        skip_group_check=True,
                    )
                    nc.tensor.matmul(
                        ps_rtr[64 * gl:64 * (gl + 1), 0:1],
                        ones_f[0:1, 0:64],
                        rtr_row[0:1, 2 * p + gl:2 * p + gl + 1],
                        start=True, stop=True, tile_position=(0, 64 * gl),
                        skip_group_check=True,
                    )
                nc.vector.tensor_copy(out=itr_col[p][:], in_=ps_itr[:])
                nc.vector.tensor_copy(out=rtr_col[p][:], in_=ps_rtr[:])
                # sigma /= trace ; P1 = 1.5 I - 0.5 sigma
                sig = PS[p][:, 128:256]
                nc.vector.tensor_scalar_mul(sig, sig, itr_col[p][:])
                nc.vector.scalar_tensor_tensor(
                    out=PS[p][:, 0:128], in0=sig, scalar=-0.5, in1=eye15[:],
                    op0=AOP.mult, op1=AOP.add)

            # Newton-Schulz iterations 2..5:
            #   [P^2 | P sigma] = P @ [P | sigma];  P' = 1.5 P - 0.5 P^2 (P sigma)
            for _ in range(ITER_NUM - 1):
                for p in range(2):
                    ps1 = ps_rot.tile([128, 256], F32, tag="rot")
                    nc.tensor.matmul(ps1[:], PS[p][:, 0:128], PS[p][:, 0:256],
                                     start=True, stop=True)
                    tmp = small.tile([128, 256], F32, tag="scratch")
                    nc.vector.tensor_copy(out=tmp[:], in_=ps1[:])
                    ps2 = ps_rot.tile([128, 128], F32, tag="rot")
                    nc.tensor.matmul(ps2[:], tmp[:, 0:128], tmp[:, 128:256],
                                     start=True, stop=True)
                    tP = small.tile([128, 128], F32, tag="tP")
                    nc.vector.tensor_scalar_mul(tP[:], PS[p][:, 0:128], 1.5)
                    nc.vector.scalar_tensor_tensor(
                        out=PS[p][:, 0:128], in0=ps2[:], scalar=-0.5,
                        in1=tP[:], op0=AOP.mult, op1=AOP.add)

            # W = (P / sqrt(tr)) * gamma_col ; bias = beta - mu^T W
            Wbf = [keep.tile([128, 128], BF16, name=f"Wbf{p}", tag=f"Wbf{p}") for p in range(2)]
            brow_f = keep.tile([1, C], F32)
            for p in range(2):
                wmf = small.tile([128, 128], F32, tag="wmf")
                nc.vector.tensor_scalar_mul(wmf[:], PS[p][:, 0:128],
                                            rtr_col[p][:])
                ps_g = ps_rot.tile([128, 128], F32, tag="rot")
                nc.tensor.matmul(ps_g[:], ones_f[0:1, 0:128],
                                 gam_row[0:1, 128 * p:128 * (p + 1)],
                                 start=True, stop=True)
                Wf = small.tile([128, 128], F32, tag="Wf")
                nc.vector.tensor_mul(Wf[:], wmf[:], ps_g[:])
                nc.vector.tensor_copy(out=Wbf[p][:], in_=Wf[:])
                ps_b = ps_rot.tile([1, 128], F32, tag="rot")
                nc.tensor.matmul(ps_b[:], mu[p][:], Wf[:],
                                 start=True, stop=True)
                nc.vector.scalar_tensor_tensor(
                    out=brow_f[0:1, 128 * p:128 * (p + 1)], in0=ps_b[:],
                    scalar=-1.0, in1=bet_row[0:1, 128 * p:128 * (p + 1)],
                    op0=AOP.mult, op1=AOP.add)
            brow_bf = keep.tile([1, C], BF16)
            nc.vector.tensor_copy(out=brow_bf[:], in_=brow_f[:])

            # --------------- pass 2: whiten ---------------
            for s in range(NSUP):
                ns = min(SUP, NCHUNK - s * SUP)
                ot = outp.tile([128, ns, C], F32, tag="ot")
                for c in range(ns):
                    k = s * SUP + c
                    if k % 2 == 0:
                        po = ps_rot.tile([128, 256], F32, tag="rot")
                    else:
                        po = ps_rot2.tile([128, 256], F32, tag="rot2")
                    nc.tensor.matmul(po[:], ones_bf[0:1, 0:128], brow_bf[:],
                                     start=True, stop=False,
                                     skip_group_check=True)
                    nc.tensor.matmul(po[:, 0:128],
                                     XtAB[:, 0, k * CHUNK:(k + 1) * CHUNK],
                                     Wbf[0][:], start=False, stop=True,
                                     skip_group_check=True)
                    nc.tensor.matmul(po[:, 128:256],
                                     XtAB[:, 1, k * CHUNK:(k + 1) * CHUNK],
                                     Wbf[1][:], start=False, stop=True,
                                     skip_group_check=True)
                    if k % 2 == 0:
                        nc.vector.tensor_copy(out=ot[:, c, :], in_=po[:])
                    else:
                        nc.scalar.copy(out=ot[:, c, :], in_=po[:])
                orows = yv[s * SUP * CHUNK:(s * SUP + ns) * CHUNK, :]
                orows = orows.rearrange("(c p) f -> p c f", p=128)
                nc.gpsimd.dma_start(out=orows[:], in_=ot[:])

    nc.finalize()
    return nc


_NC_CACHE = None


def _get_nc():
    global _NC_CACHE
    if _NC_CACHE is None:
        _NC_CACHE = build_bass()
    return _NC_CACHE


def make_in_maps(x, gamma, beta):
    x = np.ascontiguousarray(np.asarray(x, dtype=np.float32))
    gamma = np.asarray(gamma, dtype=np.float32)
    beta = np.asarray(beta, dtype=np.float32)
    eye = np.eye(128, dtype=np.float32)
    maps = []
    for i in range(NCORES):
        maps.append({
            "x": np.ascontiguousarray(x[i * BLOC:(i + 1) * BLOC]),
            "gamma": gamma,
            "beta": beta,
            "eye": eye,
        })
    return maps


def kernel(x, gamma, beta):
    nc = _get_nc()
    in_maps = make_in_maps(x, gamma, beta)
    res = run_bass_kernel_spmd(nc, in_maps, core_ids=list(range(NCORES)))
    out = np.concatenate([res.results[i]["out"] for i in range(NCORES)],
                         axis=0)
    return out.astype(np.float32)


if __name__ == "__main__":
    nc = build_bass()
    print("graph built OK")


# revision 20
# speedup vs baseline: 1.6731x; 1.0990x over previous
"""Trainium2 Bass kernel: DecorrelationNormalization (IterNorm whitening).

Input  x: (64, 56, 56, 256) f32, gamma/beta: (1,1,1,256) f32.
Sharding: data-parallel over batch across 8 NeuronCores (8 batches/core).
Per-group (4 groups of 64 channels) covariance stats are computed locally
as uncentered second moments + channel sums, all-reduced (132KB), the tiny
Newton-Schulz iteration is replicated on every core, and the whitening
matmul is applied locally from a bf16 SBUF-resident transposed cache.
"""

import sys

for p in ("/opt/trn_rl_repo", "/opt/pypackages"):
    if p not in sys.path:
        sys.path.append(p)

import numpy as np

import concourse.bass as bass
import concourse.bacc as bacc
import concourse.tile as tile
from concourse import mybir
from concourse.bass_utils import run_bass_kernel_spmd
F32 = mybir.dt.float32
BF16 = mybir.dt.bfloat16

# Problem constants (hardcoded per spec).
B, H, W, C = 64, 56, 56, 256
NCORES = 8
BLOC = B // NCORES                    # 8 batches per core
NLOC = BLOC * H * W                   # 25088 positions per core
NGLOB = B * H * W                     # 200704 positions globally
CHUNK = 128                           # positions per chunk (partition dim)
NCHUNK = NLOC // CHUNK                # 196
SUP = 5                               # chunks per super-tile (DMA batch)
NSUP = (NCHUNK + SUP - 1) // SUP      # 40 (last super has 1 chunk)
EPS = 1e-5
ITER_NUM = 5

AOP = mybir.AluOpType
AFT = mybir.ActivationFunctionType


def build_bass() -> bass.Bass:
    nc = bacc.Bacc(None, num_devices=NCORES)

    x_d = nc.declare_dram_parameter("x", [BLOC, H, W, C], F32, isOutput=False)
    g_d = nc.declare_dram_parameter("gamma", [1, 1, 1, C], F32, isOutput=False)
    b_d = nc.declare_dram_parameter("beta", [1, 1, 1, C], F32, isOutput=False)
    eye_d = nc.declare_dram_parameter("eye", [128, 128], F32, isOutput=False)
    y_d = nc.declare_dram_parameter("out", [BLOC, H, W, C], F32, isOutput=True)

    xv = x_d[:].flatten_outer_dims()     # (25088, 256)
    yv = y_d[:].flatten_outer_dims()     # (25088, 256)
    gv = g_d[:].flatten_outer_dims()     # (1, 256)
    bv = b_d[:].flatten_outer_dims()     # (1, 256)

    with tile.TileContext(nc) as tc:
        with (
            tc.tile_pool(name="keep", bufs=1) as keep,
            tc.tile_pool(name="inp", bufs=8) as inp,
            tc.tile_pool(name="btp", bufs=4) as btp,
            tc.tile_pool(name="outp", bufs=8) as outp,
            tc.tile_pool(name="small", bufs=1) as small,
            tc.tile_pool(name="ps_acc", bufs=1, space="PSUM") as ps_acc,
            tc.tile_pool(name="ps_rot", bufs=3, space="PSUM") as ps_rot,
            tc.tile_pool(name="ps_rot2", bufs=3, space="PSUM") as ps_rot2,
            tc.tile_pool(name="dram", bufs=1, space="DRAM") as dram,
        ):
            # ---------------- constants ----------------
            eye_sb = keep.tile([128, 128], F32)
            nc.sync.dma_start(out=eye_sb[:], in_=eye_d[:])
            eye_bf = keep.tile([128, 128], BF16)
            nc.vector.tensor_copy(out=eye_bf[:], in_=eye_sb[:])
            eye15 = keep.tile([128, 128], F32)
            nc.vector.tensor_scalar_mul(eye15[:], eye_sb[:], 1.5)
            ones_f = keep.tile([1, 128], F32)
            nc.vector.memset(ones_f[:], 1.0)
            ones_bf = keep.tile([1, 128], BF16)
            nc.vector.memset(ones_bf[:], 1.0)
            gam_row = keep.tile([1, C], F32)
            nc.sync.dma_start(out=gam_row[:], in_=gv[:])
            bet_row = keep.tile([1, C], F32)
            nc.sync.dma_start(out=bet_row[:], in_=bv[:])

            # bf16 transposed cache: [channel, pair, position]
            XtAB = keep.tile([128, 2, NLOC], BF16)

            # --------------- pass 1: stats + transpose cache ---------------
            # Per chunk (128 positions): cast f32->bf16, then per channel
            # half: cov matmul with an embedded ones column (accumulating
            # second moments + channel sums in PSUM), and a plain matmul
            # against the identity producing the transposed tile.
            ps_cov01 = ps_acc.tile([128, 129], F32)
            ps_cov23 = ps_acc.tile([128, 129], F32)

            for s in range(NSUP):
                ns = min(SUP, NCHUNK - s * SUP)
                it = inp.tile([128, ns, 256], F32, tag="it")
                rows = xv[s * SUP * CHUNK:(s * SUP + ns) * CHUNK, :]
                rows = rows.rearrange("(c p) f -> p c f", p=128)
                nc.gpsimd.dma_start(out=it[:], in_=rows[:])

                bt = btp.tile([128, ns, 260], BF16, tag="bt")
                nc.gpsimd.memset(bt[:, :, 128:129], 1.0)
                nc.gpsimd.memset(bt[:, :, 258:259], 1.0)
                nc.vector.tensor_scalar_mul(bt[:, :, 0:128],
                                            it[:, :, 0:128], 1.0)
                nc.vector.tensor_scalar_mul(bt[:, :, 130:258],
                                            it[:, :, 128:256], 1.0)

                for c in range(ns):
                    k = s * SUP + c
                    first = (k == 0)
                    last = (k == NCHUNK - 1)
                    t0 = bt[:, c, 0:128]
                    t1 = bt[:, c, 130:258]
                    if k % 4 == 0:
                        pot = ps_rot.tile([128, 256], F32, tag="rot")
                    else:
                        pot = ps_rot2.tile([128, 256], F32, tag="rot2")
                    nc.tensor.matmul(ps_cov01[:], t0, bt[:, c, 0:129],
                                     start=first, stop=last)
                    nc.tensor.matmul(pot[:, 0:128], t0, eye_bf[:],
                                     start=True, stop=True,
                                     skip_group_check=True)
                    nc.tensor.matmul(ps_cov23[:], t1, bt[:, c, 130:259],
                                     start=first, stop=last)
                    nc.tensor.matmul(pot[:, 128:256], t1, eye_bf[:],
                                     start=True, stop=True,
                                     skip_group_check=True)
                    dst = XtAB[:, :, k * CHUNK:(k + 1) * CHUNK]
                    if k % 4 == 0:
                        nc.vector.tensor_copy(out=dst, in_=pot[:])
                    else:
                        nc.scalar.copy(out=dst, in_=pot[:])

            # --------------- all-reduce the stats ---------------
            S_sb = keep.tile([128, 258], F32)
            nc.vector.tensor_copy(out=S_sb[:, 0:129], in_=ps_cov01[:])
            nc.vector.tensor_copy(out=S_sb[:, 129:258], in_=ps_cov23[:])

            bounce_in = dram.tile([128, 258], F32)
            bounce_out = dram.tile([128, 258], F32)
            nc.sync.dma_start(out=bounce_in[:], in_=S_sb[:])
            nc.gpsimd.collective_compute(
                "AllReduce",
                AOP.add,
                replica_groups=[list(range(NCORES))],
                ins=[bounce_in[:].opt()],
                outs=[bounce_out[:].opt()],
            )
            S_red = S_sb
            nc.sync.dma_start(out=S_red[:], in_=bounce_out[:])

            # --------------- replicated stats assembly + Newton-Schulz -----
            # Per pair: PS tile holds [P | sigma] as (128, 256).
            PS = [keep.tile([128, 256], F32, name=f"PS{p}", tag=f"PS{p}") for p in range(2)]
            mu = [keep.tile([128, 1], F32, name=f"mu{p}", tag=f"mu{p}") for p in range(2)]
            itr_col = [keep.tile([128, 1], F32, name=f"itr{p}", tag=f"itr{p}") for p in range(2)]
            rtr_col = [keep.tile([128, 1], F32, name=f"rtr{p}", tag=f"rtr{p}") for p in range(2)]
            trrow = keep.tile([1, 4], F32)

            a_coef = (1.0 - EPS) / (NGLOB - 1.0)
            b_coef = -(1.0 - EPS) * NGLOB / (NGLOB - 1.0)

            for p in range(2):
                cov = S_red[:, 129 * p:129 * p + 128]
                sums = S_red[:, 129 * p + 128:129 * p + 129]
                nc.vector.tensor_scalar_mul(mu[p][:], sums, 1.0 / NGLOB)
                # mu row via PE transpose
                ps_mur = ps_rot.tile([1, 128], F32, tag="rot")
                nc.tensor.transpose(ps_mur[:], mu[p][:], eye_sb[:])
                mur = small.tile([1, 128], F32, tag="rowtmp")
                nc.vector.tensor_copy(out=mur[:], in_=ps_mur[:])
                # mu mu^T diagonal blocks (64x64 each)
                ps_muu = ps_rot.tile([128, 64], F32, tag="rot")
                for gl in range(2):
                    nc.tensor.matmul(
                        ps_muu[64 * gl:64 * (gl + 1), 0:64],
                        mur[0:1, 64 * gl:64 * (gl + 1)],
                        mur[0:1, 64 * gl:64 * (gl + 1)],
                        start=True, stop=True,
                        tile_position=(0, 64 * gl),
                        skip_group_check=True,
                    )
                # sigma := (1-eps)*(S - N mu mu^T)/(N-1) + eps I, block-diag
                sig = PS[p][:, 128:256]
                nc.vector.memset(sig, 0.0)
                mt = small.tile([128, 64], F32, tag="mt")
                nc.vector.tensor_scalar_mul(mt[:], ps_muu[:], b_coef)
                for gl in range(2):
                    sblk = cov[64 * gl:64 * (gl + 1), 64 * gl:64 * (gl + 1)]
                    nc.vector.scalar_tensor_tensor(
                        out=PS[p][64 * gl:64 * (gl + 1),
                                  128 + 64 * gl:128 + 64 * (gl + 1)],
                        in0=sblk, scalar=a_coef,
                        in1=mt[64 * gl:64 * (gl + 1), :],
                        op0=AOP.mult, op1=AOP.add,
                    )
                nc.vector.scalar_tensor_tensor(
                    out=sig, in0=eye_sb[:], scalar=EPS, in1=sig,
                    op0=AOP.mult, op1=AOP.add)
                # traces of the two 64x64 blocks
                dt_full = small.tile([128, 256], F32, tag="scratch", name="dt_full")
                dt_ = dt_full[:, 0:128]
                nc.vector.tensor_mul(dt_, sig, eye_sb[:])
                dcol = small.tile([128, 1], F32, tag="dcol")
                nc.vector.tensor_reduce(dcol[:], dt_,
                                        axis=mybir.AxisListType.X, op=AOP.add)
                ps_dr = ps_rot.tile([1, 128], F32, tag="rot")
                nc.tensor.transpose(ps_dr[:], dcol[:], eye_sb[:])
                drow = small.tile([1, 128], F32, tag="rowtmp")
                nc.vector.tensor_copy(out=drow[:], in_=ps_dr[:])
                for gl in range(2):
                    nc.vector.tensor_reduce(
                        trrow[0:1, 2 * p + gl:2 * p + gl + 1],
                        drow[0:1, 64 * gl:64 * (gl + 1)],
                        axis=mybir.AxisListType.X, op=AOP.add)

            # 1/tr and 1/sqrt(tr) (+1 Newton-Raphson polish for rsqrt)
            itr_row = keep.tile([1, 4], F32)
            nc.vector.reciprocal(itr_row[:], trrow[:])
            rtr_row = keep.tile([1, 4], F32)
            sq_row = keep.tile([1, 4], F32)
            nc.scalar.activation(out=sq_row[:], in_=trrow[:], func=AFT.Sqrt)
            nc.vector.reciprocal(rtr_row[:], sq_row[:])
            nr = small.tile([1, 4], F32, tag="nr")
            nc.vector.tensor_mul(nr[:], rtr_row[:], rtr_row[:])
            nc.vector.tensor_mul(nr[:], nr[:], trrow[:])
            nc.vector.tensor_scalar(out=nr[:], in0=nr[:], scalar1=-0.5,
                                    scalar2=1.5, op0=AOP.mult, op1=AOP.add)
            nc.vector.tensor_mul(rtr_row[:], rtr_row[:], nr[:])

            # broadcast per-group scalars into per-partition columns
            for p in range(2):
                ps_itr = ps_rot.tile([128, 1], F32, tag="rot")
                ps_rtr = ps_rot.tile([128, 1], F32, tag="rot")
                for gl in range(2):
                    nc.tensor.matmul(
                        ps_itr[64 * gl:64 * (gl + 1), 0:1],
                        ones_f[0:1, 0:64],
                        itr_row[0:1, 2 * p + gl:2 * p + gl + 1],
                        start=True, stop=True, tile_position=(0, 64 * gl),
                        skip_group_check=True,
                    )
                    nc.tensor.matmul(
                        ps_rtr[64 * gl:64 * (gl + 1), 0:1],
                        ones_f[0:1, 0:64],
                        rtr_row[0:1, 2 * p + gl:2 * p + gl + 1],
                        start=True, stop=True, tile_position=(0, 64 * gl),
                        skip_group_check=True,
                    )
                nc.vector.tensor_copy(out=itr_col[p][:], in_=ps_itr[:])
                nc.vector.tensor_copy(out=rtr_col[p][:], in_=ps_rtr[:])
                # sigma /= trace ; P1 = 1.5 I - 0.5 sigma
                sig = PS[p][:, 128:256]
                nc.vector.tensor_scalar_mul(sig, sig, itr_col[p][:])
                nc.vector.scalar_tensor_tensor(
                    out=PS[p][:, 0:128], in0=sig, scalar=-0.5, in1=eye15[:],
                    op0=AOP.mult, op1=AOP.add)

            # Newton-Schulz iterations 2..5:
            #   [P^2 | P sigma] = P @ [P | sigma];  P' = 1.5 P - 0.5 P^2 (P sigma)
            for _ in range(ITER_NUM - 1):
                for p in range(2):
                    ps1 = ps_rot.tile([128, 256], F32, tag="rot")
                    nc.tensor.matmul(ps1[:], PS[p][:, 0:128], PS[p][:, 0:256],
                                     start=True, stop=True)
                    tmp = small.tile([128, 256], F32, tag="scratch")
                    nc.vector.tensor_copy(out=tmp[:], in_=ps1[:])
                    ps2 = ps_rot.tile([128, 128], F32, tag="rot")
                    nc.tensor.matmul(ps2[:], tmp[:, 0:128], tmp[:, 128:256],
                                     start=True, stop=True)
                    tP = small.tile([128, 128], F32, tag="tP")
                    nc.vector.tensor_scalar_mul(tP[:], PS[p][:, 0:128], 1.5)
                    nc.vector.scalar_tensor_tensor(
                        out=PS[p][:, 0:128], in0=ps2[:], scalar=-0.5,
                        in1=tP[:], op0=AOP.mult, op1=AOP.add)

            # W = (P / sqrt(tr)) * gamma_col ; bias = beta - mu^T W
            Wbf = [keep.tile([128, 128], BF16, name=f"Wbf{p}", tag=f"Wbf{p}") for p in range(2)]
            brow_f = keep.tile([1, C], F32)
            for p in range(2):
                wmf = small.tile([128, 128], F32, tag="wmf")
                nc.vector.tensor_scalar_mul(wmf[:], PS[p][:, 0:128],
                                            rtr_col[p][:])
                ps_g = ps_rot.tile([128, 128], F32, tag="rot")
                nc.tensor.matmul(ps_g[:], ones_f[0:1, 0:128],
                                 gam_row[0:1, 128 * p:128 * (p + 1)],
                                 start=True, stop=True)
                Wf = small.tile([128, 128], F32, tag="Wf")
                nc.vector.tensor_mul(Wf[:], wmf[:], ps_g[:])
                nc.vector.tensor_copy(out=Wbf[p][:], in_=Wf[:])
                ps_b = ps_rot.tile([1, 128], F32, tag="rot")
                nc.tensor.matmul(ps_b[:], mu[p][:], Wf[:],
                                 start=True, stop=True)
                nc.vector.scalar_tensor_tensor(
                    out=brow_f[0:1, 128 * p:128 * (p + 1)], in0=ps_b[:],
                    scalar=-1.0, in1=bet_row[0:1, 128 * p:128 * (p + 1)],
                    op0=AOP.mult, op1=AOP.add)
            brow_bf = keep.tile([1, C], BF16)
            nc.vector.tensor_copy(out=brow_bf[:], in_=brow_f[:])

            # --------------- pass 2: whiten ---------------
            for s in range(NSUP):
                ns = min(SUP, NCHUNK - s * SUP)
                ot = outp.tile([128, ns, C], F32, tag="ot")
                for c in range(ns):
                    k = s * SUP + c
                    if k % 2 == 0:
                        po = ps_rot.tile([128, 256], F32, tag="rot")
                    else:
                        po = ps_rot2.tile([128, 256], F32, tag="rot2")
                    nc.tensor.matmul(po[:], ones_bf[0:1, 0:128], brow_bf[:],
                                     start=True, stop=False,
                                     skip_group_check=True)
                    nc.tensor.matmul(po[:, 0:128],
                                     XtAB[:, 0, k * CHUNK:(k + 1) * CHUNK],
                                     Wbf[0][:], start=False, stop=True,
                                     skip_group_check=True)
                    nc.tensor.matmul(po[:, 128:256],
                                     XtAB[:, 1, k * CHUNK:(k + 1) * CHUNK],
                                     Wbf[1][:], start=False, stop=True,
                                     skip_group_check=True)
                    if k % 2 == 0:
                        nc.vector.tensor_copy(out=ot[:, c, :], in_=po[:])
                    else:
                        nc.scalar.copy(out=ot[:, c, :], in_=po[:])
                orows = yv[s * SUP * CHUNK:(s * SUP + ns) * CHUNK, :]
                orows = orows.rearrange("(c p) f -> p c f", p=128)
                nc.gpsimd.dma_start(out=orows[:], in_=ot[:])

    nc.finalize()
    return nc


_NC_CACHE = None


def _get_nc():
    global _NC_CACHE
    if _NC_CACHE is None:
        _NC_CACHE = build_bass()
    return _NC_CACHE


def make_in_maps(x, gamma, beta):
    x = np.ascontiguousarray(np.asarray(x, dtype=np.float32))
    gamma = np.asarray(gamma, dtype=np.float32)
    beta = np.asarray(beta, dtype=np.float32)
    eye = np.eye(128, dtype=np.float32)
    maps = []
    for i in range(NCORES):
        maps.append({
            "x": np.ascontiguousarray(x[i * BLOC:(i + 1) * BLOC]),
            "gamma": gamma,
            "beta": beta,
            "eye": eye,
        })
    return maps


def kernel(x, gamma, beta):
    nc = _get_nc()
    in_maps = make_in_maps(x, gamma, beta)
    res = run_bass_kernel_spmd(nc, in_maps, core_ids=list(range(NCORES)))
    out = np.concatenate([res.results[i]["out"] for i in range(NCORES)],
                         axis=0)
    return out.astype(np.float32)


if __name__ == "__main__":
    nc = build_bass()
    print("graph built OK")
